# revision 5
# baseline (speedup 1.0000x reference)
"""Trainium2 Bass kernel: per-(b,c) cross attention with 1x1-conv projections.

Reference computation (per batch b, channel c):
    q = Wq @ query[b] + bq   (channel-mixing 1x1 conv), same for k, v
    scores[h,g] = sum_w q[c,h,w] * k[c,g,w]
    attn = softmax(scores, axis=g)
    out[c,h,w] = sum_g attn[h,g] * v[c,g,w]

Sharding: data-parallel over batch B=8 across the 8 NeuronCores.

Per-core kernel, two phases:
  Phase 1 (conv): GEMM [256,256] x [256, 36864] for q/k/v, bias fused
    into the PSUM->SBUF copy on ACT, results spilled to DRAM scratch in
    natural [c, h*w] layout.
  Phase 2 (attention, per channel): reload per-channel [192,192] tiles
    (the DMA reload performs the c->spatial re-partitioning for free),
    PE-transpose q,k to [w, h] layout, compute scores^T[g,h] with w as
    the contraction dim, exp(s - 50) on ACT (softmax shifted by a global
    constant instead of a per-row max: |scores| <~ 85 stays inside
    fp32's exp range, and softmax is invariant to any per-row constant),
    then the AV matmul with a fused ones-column computing the softmax
    denominator, and a per-partition reciprocal scale.

Precision configs:
  FAST: fp16 inputs/weights/spilled q,k (10-bit mantissa, full PE rate),
        bf16 spilled v and attention weights (bf16 range needed: exp can
        reach ~2e14, over fp16's max).
  SAFE: everything fp32 (exact, 4 cycles/row matmuls, PE-bound).
"""

import numpy as np

B, C, H, W = 8, 256, 192, 192
HW = H * W            # 36864
NT = 384              # conv spatial tile = 2 image rows
NTILES = HW // NT     # 96
N_CORES = 8
EXP_BIAS = -50.0

FAST = {'x': 'f16', 'qk': 'f16', 'v': 'bf16'}
SAFE = {'x': 'f32', 'qk': 'f32', 'v': 'f32'}
CFG = FAST

_CACHE = {}


def _np_dt(d):
    import ml_dtypes
    return {'f16': np.float16, 'bf16': ml_dtypes.bfloat16,
            'f32': np.float32}[d]


def _build(cfg_key):
    import concourse.bacc as bacc
    import concourse.tile as tile
    import concourse.mybir as mybir

    cfg = dict(zip(('x', 'qk', 'v'), cfg_key))
    f32 = mybir.dt.float32
    DT = {'f16': mybir.dt.float16, 'bf16': mybir.dt.bfloat16, 'f32': f32}
    xdt = DT[cfg['x']]       # conv inputs + weights
    qkdt = DT[cfg['qk']]     # spilled q,k; scores matmul operands
    vdt = DT[cfg['v']]       # spilled v, exp output; AV matmul operands
    AF = mybir.ActivationFunctionType

    nc = bacc.Bacc('TRN2', target_bir_lowering=False, debug=False,
                   enable_asserts=False, num_devices=N_CORES)

    xq = nc.dram_tensor('xq', [C, HW], xdt, kind='ExternalInput').ap()
    xk = nc.dram_tensor('xk', [C, HW], xdt, kind='ExternalInput').ap()
    xv = nc.dram_tensor('xv', [C, HW], xdt, kind='ExternalInput').ap()
    wqT = nc.dram_tensor('wqT', [C, C], xdt, kind='ExternalInput').ap()
    wkT = nc.dram_tensor('wkT', [C, C], xdt, kind='ExternalInput').ap()
    wvT = nc.dram_tensor('wvT', [C, C], xdt, kind='ExternalInput').ap()
    bq = nc.dram_tensor('bq', [C, 1], f32, kind='ExternalInput').ap()
    bk = nc.dram_tensor('bk', [C, 1], f32, kind='ExternalInput').ap()
    bv = nc.dram_tensor('bv', [C, 1], f32, kind='ExternalInput').ap()
    ident = nc.dram_tensor('ident', [128, 128], qkdt, kind='ExternalInput').ap()
    out = nc.dram_tensor('out', [C, HW], f32, kind='ExternalOutput').ap()

    # DRAM scratch for conv outputs
    qd = nc.dram_tensor('q_scratch', [C, HW], qkdt).ap()
    kd = nc.dram_tensor('k_scratch', [C, HW], qkdt).ap()
    vd = nc.dram_tensor('v_scratch', [C, HW], vdt).ap()

    q3 = qd.rearrange('c (h w) -> c h w', w=W)
    k3 = kd.rearrange('c (h w) -> c h w', w=W)
    v3 = vd.rearrange('c (h w) -> c h w', w=W)
    out3 = out.rearrange('c (h w) -> c h w', w=W)

    with tile.TileContext(nc) as tc:
        with tc.tile_pool(name='const', bufs=1) as const:
            ident_sb = const.tile([128, 128], qkdt, tag='ident')
            nc.sync.dma_start(ident_sb[:], ident[:])
            expb = const.tile([128, 1], f32, tag='expb')
            nc.gpsimd.memset(expb[:], EXP_BIAS)
            w_sb = {}
            b_sb = {}
            for name, wap, bap in (('q', wqT, bq), ('k', wkT, bk), ('v', wvT, bv)):
                for ic in range(2):
                    t = const.tile([128, C], xdt, tag=f'w{name}{ic}')
                    nc.sync.dma_start(t[:], wap[ic * 128:(ic + 1) * 128, :])
                    w_sb[name, ic] = t
                for ct in range(2):
                    t = const.tile([128, 1], f32, tag=f'b{name}{ct}')
                    nc.sync.dma_start(t[:], bap[ct * 128:(ct + 1) * 128, :])
                    b_sb[name, ct] = t

            # ---------------- Phase 1: conv GEMMs ----------------
            with tc.tile_pool(name='cv', bufs=4) as cv, \
                 tc.tile_pool(name='cvp', bufs=4, space='PSUM') as cvp:
                for name, xap, dap, odt in (
                        ('q', xq, qd, qkdt), ('k', xk, kd, qkdt),
                        ('v', xv, vd, vdt)):
                    for t in range(NTILES):
                        sl = slice(t * NT, (t + 1) * NT)
                        xt = []
                        for ic in range(2):
                            xti = cv.tile([128, NT], xdt, tag='xin')
                            nc.sync.dma_start(
                                xti[:], xap[ic * 128:(ic + 1) * 128, sl])
                            xt.append(xti)
                        for ct in range(2):
                            csl = slice(ct * 128, (ct + 1) * 128)
                            ps = cvp.tile([128, NT], f32, tag='cps')
                            for ic in range(2):
                                nc.tensor.matmul(
                                    ps[:], w_sb[name, ic][:, csl], xt[ic][:],
                                    start=(ic == 0), stop=(ic == 1))
                            ot = cv.tile([128, NT], odt, tag='cout')
                            # PSUM -> SBUF (+ cast) with channel bias fused
                            nc.scalar.activation(
                                ot[:], ps[:], AF.Identity,
                                bias=b_sb[name, ct][:])
                            nc.sync.dma_start(dap[csl, sl], ot[:])

            # ---------------- Phase 2: attention per channel ----------------
            with tc.tile_pool(name='at', bufs=3) as at, \
                 tc.tile_pool(name='tps', bufs=4, space='PSUM') as tps, \
                 tc.tile_pool(name='scp', bufs=2, space='PSUM') as scp, \
                 tc.tile_pool(name='avp', bufs=2, space='PSUM') as avp:
                for c in range(C):
                    # ---- load q, k natural [h, w]; v natural [g, w] ----
                    qn0 = at.tile([128, W], qkdt, tag='qn0')
                    nc.sync.dma_start(qn0[:], q3[c, 0:128, :])
                    qn1 = at.tile([128, W], qkdt, tag='qn1')
                    nc.sync.dma_start(qn1[0:64, :], q3[c, 128:192, :])
                    kn0 = at.tile([128, W], qkdt, tag='kn0')
                    nc.sync.dma_start(kn0[:], k3[c, 0:128, :])
                    kn1 = at.tile([128, W], qkdt, tag='kn1')
                    nc.sync.dma_start(kn1[0:64, :], k3[c, 128:192, :])
                    # v straight into the AV rhs tile; ones column at 192
                    va = at.tile([128, 224], vdt, tag='va')
                    nc.sync.dma_start(va[:, 0:192], v3[c, 0:128, :])
                    nc.gpsimd.memset(va[:, 192:193], 1.0)
                    vb = at.tile([128, 224], vdt, tag='vb')
                    nc.sync.dma_start(vb[0:64, 0:192], v3[c, 128:192, :])
                    nc.gpsimd.memset(vb[0:64, 192:193], 1.0)

                    # ---- PE transposes: psA = w 0:128, psB = w 128:192 ----
                    # cols 0:192 = qT (h), cols 192:384 = kT (g)
                    psA = tps.tile([128, 384], qkdt, tag='tps')
                    psB = tps.tile([128, 384], qkdt, tag='tps')
                    for src0, src1, off in ((qn0, qn1, 0), (kn0, kn1, 192)):
                        nc.tensor.transpose(
                            psA[:, off:off + 128], src0[:, 0:128], ident_sb[:])
                        nc.tensor.transpose(
                            psA[:, off + 128:off + 192], src1[0:64, 0:128],
                            ident_sb[0:64, 0:64])
                        nc.tensor.transpose(
                            psB[0:64, off:off + 128], src0[:, 128:192],
                            ident_sb[:])
                        nc.tensor.transpose(
                            psB[0:64, off + 128:off + 192], src1[0:64, 128:192],
                            ident_sb[0:64, 0:64])

                    # ---- PSUM -> SBUF copies of the transposes ----
                    qTs0 = at.tile([128, 192], qkdt, tag='qTs0')
                    nc.vector.tensor_copy(qTs0[:], psA[:, 0:192])
                    qTs1 = at.tile([128, 192], qkdt, tag='qTs1')
                    nc.vector.tensor_copy(qTs1[0:64, :], psB[0:64, 0:192])
                    kTs0 = at.tile([128, 192], qkdt, tag='kTs0')
                    nc.scalar.activation(kTs0[:], psA[:, 192:384], AF.Identity)
                    kTs1 = at.tile([128, 192], qkdt, tag='kTs1')
                    nc.scalar.activation(
                        kTs1[0:64, :], psB[0:64, 192:384], AF.Identity)

                    # ---- scores^T[g, h] = sum_w kT[w,g] * qT[w,h] ----
                    sc = scp.tile([128, 512], f32, tag='sc')
                    for gt in range(2):
                        gM = 128 if gt == 0 else 64
                        gsl = slice(gt * 128, gt * 128 + gM)
                        reg = sc[0:gM, gt * 256:gt * 256 + 192]
                        nc.tensor.matmul(
                            reg, kTs0[:, gsl], qTs0[:], start=True, stop=False)
                        nc.tensor.matmul(
                            reg, kTs1[0:64, gsl], qTs1[0:64, :],
                            start=False, stop=True)

                    # ---- P^T = exp(scores^T - 50) ----
                    pt0 = at.tile([128, 192], vdt, tag='pt0')
                    nc.scalar.activation(
                        pt0[:], sc[:, 0:192], AF.Exp, bias=expb[:])
                    pt1 = at.tile([128, 192], vdt, tag='pt1')
                    nc.scalar.activation(
                        pt1[0:64, :], sc[0:64, 256:448], AF.Exp,
                        bias=expb[0:64, :])

                    # ---- out^u[h,w] = sum_g PT[g,h] v[g,w]; col 192 = sum_g PT ----
                    av = avp.tile([128, 512], f32, tag='av')
                    for ht in range(2):
                        hM = 128 if ht == 0 else 64
                        hsl = slice(ht * 128, ht * 128 + hM)
                        reg = av[0:hM, ht * 256:ht * 256 + 193]
                        nc.tensor.matmul(
                            reg, pt0[:, hsl], va[:, 0:193],
                            start=True, stop=False)
                        nc.tensor.matmul(
                            reg, pt1[0:64, hsl], vb[0:64, 0:193],
                            start=False, stop=True)

                    # ---- normalize by softmax denominator, store ----
                    rc = at.tile([128, 2], f32, tag='rc')
                    nc.vector.reciprocal(rc[:, 0:1], av[:, 192:193])
                    nc.vector.reciprocal(rc[0:64, 1:2], av[0:64, 448:449])
                    ot0 = at.tile([128, 192], f32, tag='ot0')
                    nc.vector.tensor_scalar_mul(ot0[:], av[:, 0:192], rc[:, 0:1])
                    ot1 = at.tile([128, 192], f32, tag='ot1')
                    nc.vector.tensor_scalar_mul(
                        ot1[0:64, :], av[0:64, 256:448], rc[0:64, 1:2])
                    nc.sync.dma_start(out3[c, 0:128, :], ot0[:])
                    nc.sync.dma_start(out3[c, 128:192, :], ot1[0:64, :])

    nc.compile()
    return nc


def _get_nc(cfg_key):
    if cfg_key not in _CACHE:
        _CACHE[cfg_key] = _build(cfg_key)
    return _CACHE[cfg_key]


def _in_maps(inputs, cfg):
    xdt = _np_dt(cfg['x'])
    qkdt = _np_dt(cfg['qk'])
    q = np.asarray(inputs['query'], dtype=np.float32).astype(xdt)
    k = np.asarray(inputs['key'], dtype=np.float32).astype(xdt)
    v = np.asarray(inputs['value'], dtype=np.float32).astype(xdt)
    wqT = np.ascontiguousarray(
        np.asarray(inputs['Wq'], dtype=np.float32).T).astype(xdt)
    wkT = np.ascontiguousarray(
        np.asarray(inputs['Wk'], dtype=np.float32).T).astype(xdt)
    wvT = np.ascontiguousarray(
        np.asarray(inputs['Wv'], dtype=np.float32).T).astype(xdt)
    bq = np.asarray(inputs['bq'], dtype=np.float32).reshape(C, 1)
    bk = np.asarray(inputs['bk'], dtype=np.float32).reshape(C, 1)
    bv = np.asarray(inputs['bv'], dtype=np.float32).reshape(C, 1)
    ident = np.eye(128, dtype=np.float32).astype(qkdt)
    maps = []
    for b in range(B):
        maps.append({
            'xq': q[b].reshape(C, HW), 'xk': k[b].reshape(C, HW),
            'xv': v[b].reshape(C, HW),
            'wqT': wqT, 'wkT': wkT, 'wvT': wvT,
            'bq': bq, 'bk': bk, 'bv': bv,
            'ident': ident,
        })
    return maps


def run(inputs, cfg=None, trace=False):
    """Run on hardware; returns (output [B,C,H,W], BassKernelResults)."""
    from concourse.bass_utils import run_bass_kernel_spmd
    cfg = cfg or CFG
    cfg_key = (cfg['x'], cfg['qk'], cfg['v'])
    nc = _get_nc(cfg_key)
    res = run_bass_kernel_spmd(nc, _in_maps(inputs, cfg),
                               list(range(N_CORES)), trace=trace)
    outs = np.stack(
        [res.results[b]['out'].reshape(C, H, W) for b in range(B)], axis=0)
    return outs, res


def kernel(**inputs):
    outs, _ = run(inputs)
    return outs
